# revision 32
# baseline (speedup 1.0000x reference)
"""Trainium2 Bass kernel for nn_AttentionLayer (B=8, N=1024, D=1024, H=16).

Sharding: data-parallel over batch -- one batch element per NeuronCore (8 cores).

Per-core pipeline (restructured for cross-phase overlap; bf16 operands, fp32 PSUM):
  A:  LayerNorm x / source in natural layout, PE-transpose -> snT/xnT [D, tok] bf16.
  V:  v natural via xnT-stationary matmul, ones column per head (early, so the
      attention sweep can consume groups as soon as q/k slices appear).
  Sweep hf=0 (head-pair inner): per pair p: k-proj td=p, q-proj td=p, then
      scoresT -> exp (ACT, the critical engine) -> attn@v with ones-column
      denominator -> normalize via DRAM-bounce reciprocal broadcast.
  Sweep hf=1: same groups (q/k done), with out-projection token tiles 0..3
      interleaved as PE filler; D nt4..7 at the tail.
  Final LN: rstd = Exp(-0.5*Ln(var+eps)) -- stays in the ln/exp ACT table set,
      no table thrash against the sweep's exp; apply = two scalar_tensor_tensor.
"""

import numpy as np

import concourse.bass as bass
import concourse.tile as tile
from concourse import bacc, mybir
from concourse.bass_utils import run_bass_kernel_spmd
from concourse.masks import make_identity

F32 = mybir.dt.float32
BF16 = mybir.dt.bfloat16
MM_DT = BF16   # weights / qT / kT / aoT
EPS_DT = BF16  # exp output + v_sb (attnv operands)

B, N, D, H = 8, 1024, 1024, 16
DH = D // H  # 64
EPS = 1e-5
THETA = 10000.0
NT = N // 128  # 8 token tiles
DT = D // 128  # 8 channel tiles
SCALE = float(DH) ** -0.5

_PAIRSWAP = []
for _i in range(16):
    _PAIRSWAP += [2 * _i + 1, 2 * _i]


def build_program():
    nc = bacc.Bacc("TRN2", target_bir_lowering=False, debug=False)

    x_d = nc.dram_tensor("x", [N, D], F32, kind="ExternalInput").ap()
    s_d = nc.dram_tensor("src", [N, D], F32, kind="ExternalInput").ap()
    wq_d = nc.dram_tensor("wq", [D, D], MM_DT, kind="ExternalInput").ap()
    wk_d = nc.dram_tensor("wk", [D, D], MM_DT, kind="ExternalInput").ap()
    wv_d = nc.dram_tensor("wv", [D, D], MM_DT, kind="ExternalInput").ap()
    wo_d = nc.dram_tensor("wo", [D, D], MM_DT, kind="ExternalInput").ap()
    cosf_d = nc.dram_tensor("cosf", [128, N], MM_DT, kind="ExternalInput").ap()
    sinf_d = nc.dram_tensor("sinf", [128, N], MM_DT, kind="ExternalInput").ap()
    # bias tables [128, DT] (col t = do-tile t)
    bqa_d = nc.dram_tensor("bqa", [128, DT], F32, kind="ExternalInput").ap()
    bka_d = nc.dram_tensor("bka", [128, DT], F32, kind="ExternalInput").ap()
    # row vectors for partition-broadcast loads: bv', ln_g, ln_b
    rows_d = nc.dram_tensor("rows", [3, D], F32, kind="ExternalInput").ap()

    out_d = nc.dram_tensor("out", [N, D], F32, kind="ExternalOutput").ap()
    # bounce rows: [idx] raw sums, [32+idx] reciprocals (reshaped 128-wide)
    rb_d = nc.dram_tensor("rbounce", [H * 4, 512], F32).ap()

    def bcast_row(r):
        return bass.AP(
            tensor=rows_d.tensor, offset=rows_d.offset + r * D, ap=[[0, 128], [1, D]]
        )

    with tile.TileContext(nc) as tc:
        # ---------- pools ----------
        # left stack: whole-kernel lifetime
        const = tc.alloc_tile_pool(name="const", bufs=1, side="left")
        stp = tc.alloc_tile_pool(name="stp", bufs=4, side="left")
        mvp = tc.alloc_tile_pool(name="mvp", bufs=4, side="left")
        qkv = tc.alloc_tile_pool(name="qkv", bufs=1, side="left")
        psP = tc.alloc_tile_pool(name="psP", bufs=2, space="PSUM", side="left")
        # right stack: phase-scoped (alloc in reverse release order)
        wbp = tc.alloc_tile_pool(name="wbp", bufs=16, side="right")
        rope = tc.alloc_tile_pool(name="rope", bufs=2, side="right")
        ntp = tc.alloc_tile_pool(name="ntp", bufs=1, side="right")
        xnp = tc.alloc_tile_pool(name="xnp", bufs=3, side="right")
        ldp = tc.alloc_tile_pool(name="ldp", bufs=4, side="right")
        psT = tc.alloc_tile_pool(name="psT", bufs=4, space="PSUM", side="right")

        # ---- constants (DMAs issued after the first x tiles; see below)
        ident = const.tile([128, 128], F32)
        make_identity(nc, ident)
        eps_t = const.tile([128, 1], F32)
        nc.vector.memset(eps_t, EPS)
        ones128 = const.tile([128, 128], F32)
        nc.vector.memset(ones128, 1.0)
        cosf = const.tile([128, N], MM_DT)
        sinf = const.tile([128, N], MM_DT)
        bqa = const.tile([128, DT], F32)
        bka = const.tile([128, DT], F32)
        bv_bc = const.tile([128, D], F32)

        # ---- persistent attention operands
        qT = qkv.tile([128, DT, N], MM_DT, tag="qT")
        kT = qkv.tile([128, DT, N], MM_DT, tag="kT")
        v_sb = qkv.tile([128, NT, H, 65], EPS_DT, tag="v_sb")
        aoT = qkv.tile([128, DT, N], MM_DT, tag="aoT")
        nc.vector.tensor_copy(
            v_sb[:, :, :, 64:65],
            ones128.rearrange("p (a b c) -> p a b c", a=NT, b=H, c=1),
        )

        def ln_tile(xt, out_ap):
            """LayerNorm [128, D] over free dim -> out_ap (pre-norm phase, Sqrt set)."""
            stats = stp.tile([128, 2, 6], F32, tag="stats")
            for g in range(2):
                nc.vector.bn_stats(stats[:, g, :], xt[:, g * 512:(g + 1) * 512])
            mv = mvp.tile([128, 2], F32, tag="mv")
            nc.vector.bn_aggr(mv, stats)
            nc.scalar.activation(
                mv[:, 1:2], mv[:, 1:2], mybir.ActivationFunctionType.Sqrt, bias=eps_t
            )
            nc.vector.reciprocal(mv[:, 1:2], mv[:, 1:2])
            nc.vector.tensor_scalar(
                out=out_ap,
                in0=xt,
                scalar1=mv[:, 0:1],
                scalar2=mv[:, 1:2],
                op0=mybir.AluOpType.subtract,
                op1=mybir.AluOpType.mult,
            )

        def ln_transpose(src_ap, dstT):
            """dstT [128, DT, N] bf16 = transpose of LN(src)."""
            for t in range(NT):
                xt = ldp.tile([128, D], F32, tag="ld")
                nc.sync.dma_start(xt, src_ap[t * 128:(t + 1) * 128, :])
                xn = xnp.tile([128, D], F32, tag="xn")
                ln_tile(xt, xn)
                for d in range(DT):
                    pt = psT.tile([128, 128], F32, tag="psT")
                    nc.tensor.transpose(pt, xn[:, d * 128:(d + 1) * 128], ident)
                    nc.scalar.copy(dstT[:, d, t * 128:(t + 1) * 128], pt)

        def load_w(w_d):
            tiles = []
            for dk in range(DT):
                wt = wbp.tile([128, D], MM_DT, tag="w")
                nc.sync.dma_start(wt, w_d[dk * 128:(dk + 1) * 128, :])
                tiles.append(wt)
            return tiles

        def qk_proj_td(w_tiles, srcT, dstT, ba, td, hfs=(0, 1)):
            """dstT[:, td, hfs] = RoPE(W.T @ srcT + bias) for one do-tile.

            u = psum + bias (partition pair-swap of u carries the bias swap);
            out = u*cos + swap(u)*sin, all-bf16 on DVE after the biased copy.
            """
            for hf in hfs:
                ns = slice(hf * 512, (hf + 1) * 512)
                ps = psP.tile([128, 512], F32, tag="psP")
                for dk in range(DT):
                    nc.tensor.matmul(
                        ps,
                        w_tiles[dk][:, td * 128:(td + 1) * 128],
                        srcT[:, dk, ns],
                        start=(dk == 0),
                        stop=(dk == DT - 1),
                    )
                u = rope.tile([128, 512], MM_DT, tag="u")
                nc.vector.tensor_scalar(
                    out=u, in0=ps, scalar1=ba[:, td:td + 1], scalar2=None,
                    op0=mybir.AluOpType.add,
                )
                t1 = rope.tile([128, 512], MM_DT, tag="t1")
                nc.vector.tensor_mul(t1, u, cosf[:, ns])
                qs = rope.tile([128, 512], MM_DT, tag="qs")
                nc.vector.stream_shuffle(qs, u, _PAIRSWAP)
                t2 = rope.tile([128, 512], MM_DT, tag="t2")
                nc.vector.tensor_mul(t2, qs, sinf[:, ns])
                nc.vector.tensor_add(dstT[:, td, ns], t1, t2)

        # ============ Prologue: x & src LN/transpose with V interleaved ============
        # V group t depends only on x-tile t's transposes; src tiles alternate so
        # the DVE LayerNorm pipeline feeds the PE without gaps.
        xnT = ntp.tile([128, DT, N], MM_DT, tag="xnT")
        snT = ntp.tile([128, DT, N], MM_DT, tag="snT")
        xts, sts = [], []
        wv_t = wk_t = None
        for t in range(NT):
            xt = ldp.tile([128, D], F32, tag="ld")
            nc.sync.dma_start(xt, x_d[t * 128:(t + 1) * 128, :])
            xts.append(xt)
            st = ldp.tile([128, D], F32, tag="ld")
            nc.sync.dma_start(st, s_d[t * 128:(t + 1) * 128, :])
            sts.append(st)
            if t == 1:
                nc.sync.dma_start(cosf, cosf_d)
                nc.sync.dma_start(sinf, sinf_d)
                nc.sync.dma_start(bqa, bqa_d)
                nc.sync.dma_start(bka, bka_d)
                nc.sync.dma_start(bv_bc, bcast_row(0))
                wv_t = load_w(wv_d)
            if t == 3:
                wk_t = load_w(wk_d)

        def transpose_tile(xn, dstT, t):
            for d in range(DT):
                pt = psT.tile([128, 128], F32, tag="psT")
                nc.tensor.transpose(pt, xn[:, d * 128:(d + 1) * 128], ident)
                nc.scalar.copy(dstT[:, d, t * 128:(t + 1) * 128], pt)

        for t in range(NT):
            xn = xnp.tile([128, D], F32, tag="xn")
            ln_tile(xts[t], xn)
            transpose_tile(xn, xnT, t)
            for hf in range(2):
                ds_ = slice(hf * 512, (hf + 1) * 512)
                ps = psP.tile([128, 512], F32, tag="psP")
                for dk in range(DT):
                    nc.tensor.matmul(
                        ps,
                        xnT[:, dk, t * 128:(t + 1) * 128],
                        wv_t[dk][:, ds_],
                        start=(dk == 0),
                        stop=(dk == DT - 1),
                    )
                nc.vector.tensor_add(
                    v_sb[:, t, 8 * hf:8 * hf + 8, 0:64],
                    ps.rearrange("p (j d) -> p j d", j=8),
                    bv_bc[:, ds_].rearrange("p (j d) -> p j d", j=8),
                )
            sn = xnp.tile([128, D], F32, tag="xn")
            ln_tile(sts[t], sn)
            transpose_tile(sn, snT, t)

        wq_t = load_w(wq_d)  # reuses wv slots after V completes
        # q/k projections for the first two pairs (k both halves, q half 0)
        for pr in range(2):
            qk_proj_td(wk_t, xnT, kT, bka, pr, (0, 1))
            qk_proj_td(wq_t, snT, qT, bqa, pr, (0,))
        for p in (psT, ldp, xnp):
            p.release()

        psE = tc.alloc_tile_pool(name="psE", bufs=2, space="PSUM", side="right")
        psO = tc.alloc_tile_pool(name="psO", bufs=2, space="PSUM", side="right")
        epsp = tc.alloc_tile_pool(name="epsp", bufs=16, side="left")
        att = tc.alloc_tile_pool(name="att", bufs=3, side="left")

        def attn_scores(pair, hf, lo, hi, eps_tiles):
            ns = slice(hf * 512, (hf + 1) * 512)
            for mb in range(lo, hi):
                pse = psE.tile([128, 1024], F32, tag="psE")
                nc.tensor.matmul(
                    pse[:, 0:512],
                    kT[0:64, pair, mb * 128:(mb + 1) * 128],
                    qT[0:64, pair, ns],
                    start=True, stop=True,
                )
                nc.tensor.matmul(
                    pse[:, 512:1024],
                    kT[64:128, pair, mb * 128:(mb + 1) * 128],
                    qT[64:128, pair, ns],
                    start=True, stop=True,
                )
                et = epsp.tile([128, 2, 512], EPS_DT, tag="eps")
                nc.scalar.activation(
                    et, pse, mybir.ActivationFunctionType.Exp, scale=SCALE,
                )
                eps_tiles.append(et)

        def attn_av_norm(pair, hf, eps_tiles, mid=None, fast=False):
            he, ho = 2 * pair, 2 * pair + 1
            ns = slice(hf * 512, (hf + 1) * 512)
            pso_e = psO.tile([128, 512], F32, tag="psO")
            pso_o = psO.tile([128, 512], F32, tag="psO")
            for mb in range(NT):
                e_mb = eps_tiles[mb]
                nc.tensor.matmul(
                    pso_e[0:65, :], v_sb[:, mb, he, :], e_mb[:, 0, :],
                    start=(mb == 0), stop=(mb == NT - 1),
                )
                nc.tensor.matmul(
                    pso_o[0:65, :], v_sb[:, mb, ho, :], e_mb[:, 1, :],
                    start=(mb == 0), stop=(mb == NT - 1),
                )
                if mid is not None and mb in (1, 3, 5):
                    mid((mb - 1) // 2)
            # normalize: merged-parity reciprocal via DRAM-bounce broadcast
            # (one spread/recip round trip for both heads; queue alternates)
            dq = nc.gpsimd if (pair + hf) % 2 == 0 else nc.sync
            dq2 = nc.sync if (pair + hf) % 2 == 0 else nc.gpsimd
            idx = 4 * pair + 2 * hf  # rows idx, idx+1
            r_e = att.tile([128, 512], F32, tag="r_sb")
            nc.vector.tensor_copy(r_e[0:65, :], pso_e[0:65, :])
            r_o = att.tile([128, 512], F32, tag="r_sb2")
            nc.vector.tensor_copy(r_o[0:65, :], pso_o[0:65, :])
            dq.dma_start(rb_d[idx:idx + 1, :], r_e[64:65, :])
            dq.dma_start(rb_d[idx + 1:idx + 2, :], r_o[64:65, :])
            rt = att.tile([128, 8], F32, tag="rt")
            dq.dma_start(
                rt,
                bass.AP(
                    tensor=rb_d.tensor,
                    offset=rb_d.offset + idx * 512,
                    ap=[[8, 128], [1, 8]],
                ),
            )
            nc.vector.reciprocal(rt, rt)
            dq.dma_start(
                bass.AP(
                    tensor=rb_d.tensor,
                    offset=rb_d.offset + (32 + idx) * 512,
                    ap=[[8, 128], [1, 8]],
                ),
                rt,
            )
            bc_e = att.tile([64, 512], F32, tag="bc")
            dq.dma_start(
                bc_e,
                bass.AP(
                    tensor=rb_d.tensor,
                    offset=rb_d.offset + (32 + idx) * 512,
                    ap=[[0, 64], [1, 512]],
                ),
            )
            bc_o = att.tile([64, 512], F32, tag="bc2")
            dq.dma_start(
                bc_o,
                bass.AP(
                    tensor=rb_d.tensor,
                    offset=rb_d.offset + (33 + idx) * 512,
                    ap=[[0, 64], [1, 512]],
                ),
            )
            nc.gpsimd.tensor_mul(aoT[0:64, pair, ns], r_e[0:64, :], bc_e)
            tmp = att.tile([64, 512], MM_DT, tag="tmp")
            nc.gpsimd.tensor_mul(tmp, r_o[0:64, :], bc_o)
            dq2.dma_start(aoT[64:128, pair, ns], tmp)

        wop = finp = psD = None
        wo_t = []
        g_bc = b_bc = None

        def late_pools():
            nonlocal wop, finp, psD, g_bc, b_bc
            for p in (ntp, rope, wbp):
                p.release()
            psP.release()
            wop = tc.alloc_tile_pool(name="wop", bufs=8, side="right")
            finp = tc.alloc_tile_pool(name="finp", bufs=3, side="right")
            psD = tc.alloc_tile_pool(name="psD", bufs=2, space="PSUM", side="left")
            for dk in range(DT):
                wt = wop.tile([128, D], MM_DT, tag="wo")
                nc.sync.dma_start(wt, wo_d[dk * 128:(dk + 1) * 128, :])
                wo_t.append(wt)
            g_bc = wop.tile([128, D], F32, tag="g_bc", bufs=1)
            nc.sync.dma_start(g_bc, bcast_row(1))
            b_bc = wop.tile([128, D], F32, tag="b_bc", bufs=1)
            nc.sync.dma_start(b_bc, bcast_row(2))

        def out_proj(nt):
            halves = []
            for hf in range(2):
                ds_ = slice(hf * 512, (hf + 1) * 512)
                ps = psD.tile([128, 512], F32, tag="psD")
                for dk in range(DT):
                    nc.tensor.matmul(
                        ps,
                        aoT[:, dk, nt * 128:(nt + 1) * 128],
                        wo_t[dk][:, ds_],
                        start=(dk == 0),
                        stop=(dk == DT - 1),
                    )
                fin = finp.tile([128, 512], F32, tag="t")
                nc.vector.tensor_copy(fin, ps)
                halves.append(fin)
            stats = stp.tile([128, 2, 6], F32, tag="stats")
            for g in range(2):
                nc.vector.bn_stats(stats[:, g, :], halves[g])
            mv = mvp.tile([128, 2], F32, tag="mv")
            nc.vector.bn_aggr(mv, stats)
            # rstd = 1/sqrt(var+eps) DVE-only (Newton sqrt; no ACT table switch)
            w_t = mvp.tile([128, 1], F32, tag="w")
            nc.vector.tensor_scalar(
                out=w_t, in0=mv[:, 1:2], scalar1=EPS, scalar2=None,
                op0=mybir.AluOpType.add,
            )
            hw_t = mvp.tile([128, 1], F32, tag="hw")
            nc.vector.tensor_scalar_mul(hw_t, w_t, 0.5)
            s_t = mvp.tile([128, 1], F32, tag="s")
            nc.vector.tensor_scalar_max(s_t, w_t, 0.03)
            rs_t = mvp.tile([128, 1], F32, tag="rs")
            d_t = mvp.tile([128, 1], F32, tag="d")
            for _ in range(4):
                nc.vector.reciprocal(rs_t, s_t)
                nc.vector.tensor_mul(d_t, hw_t, rs_t)
                nc.vector.scalar_tensor_tensor(
                    out=s_t, in0=s_t, scalar=0.5, in1=d_t,
                    op0=mybir.AluOpType.mult, op1=mybir.AluOpType.add,
                )
            nc.vector.reciprocal(mv[:, 1:2], s_t)
            z = finp.tile([128, D], F32, tag="z")
            for g in range(2):
                ds_ = slice(g * 512, (g + 1) * 512)
                t = finp.tile([128, 512], F32, tag="t")
                nc.vector.scalar_tensor_tensor(
                    out=t, in0=halves[g], scalar=mv[:, 0:1], in1=g_bc[:, ds_],
                    op0=mybir.AluOpType.subtract, op1=mybir.AluOpType.mult,
                )
                nc.vector.scalar_tensor_tensor(
                    out=z[:, ds_], in0=t, scalar=mv[:, 1:2], in1=b_bc[:, ds_],
                    op0=mybir.AluOpType.mult, op1=mybir.AluOpType.add,
                )
            nc.sync.dma_start(out_d[nt * 128:(nt + 1) * 128, :], z)

        # ============ Pipelined group sweep (hf0/hf1 interleaved) ============
        # Groups ordered so projection fillers spread evenly; scores for group
        # g+1 are injected inside g's attnv chain so the exp stream never
        # drains. Out-projection token tiles ride the last four iterations.
        order = [(0, 0), (1, 0), (2, 0), (3, 0), (0, 1), (4, 0), (1, 1), (5, 0),
                 (2, 1), (6, 0), (3, 1), (7, 0), (4, 1), (5, 1), (6, 1), (7, 1)]
        fillers = {
            0: [("k0", 2), ("k1", 2), ("q0", 2)],
            1: [("k0", 3), ("k1", 3), ("q0", 3)],
            2: [("q1", 0), ("k0", 4)],
            3: [("k1", 4), ("q0", 4)],
            4: [("k0", 5), ("q1", 1)],
            5: [("k1", 5), ("q0", 5)],
            6: [("k0", 6), ("q1", 2)],
            7: [("k1", 6), ("q0", 6)],
            8: [("k0", 7), ("q1", 3)],
            9: [("k1", 7), ("q0", 7)],
            10: [("q1", 4), ("q1", 5)],
            11: [("q1", 6), ("q1", 7)],
        }
        douts = {12: 0, 13: 1, 14: 2, 15: 3}
        store = {}
        store[order[0]] = []
        attn_scores(order[0][0], order[0][1], 0, NT, store[order[0]])
        for i, g in enumerate(order):
            nxt = order[i + 1] if i + 1 < len(order) else None
            if nxt is not None:
                store[nxt] = []

                def mid(j, nx=nxt):
                    attn_scores(nx[0], nx[1], j, j + 1, store[nx])
            else:
                mid = None
            attn_av_norm(g[0], g[1], store.pop(g), mid=mid, fast=(g == (7, 1)))
            if nxt is not None:
                attn_scores(nxt[0], nxt[1], 3, NT, store[nxt])
            for kind, td in fillers.get(i, ()):
                if kind == "k0":
                    qk_proj_td(wk_t, xnT, kT, bka, td, (0,))
                elif kind == "k1":
                    qk_proj_td(wk_t, xnT, kT, bka, td, (1,))
                elif kind == "q0":
                    qk_proj_td(wq_t, snT, qT, bqa, td, (0,))
                else:
                    qk_proj_td(wq_t, snT, qT, bqa, td, (1,))
            if i == 11:
                late_pools()
            if i in douts:
                out_proj(douts[i])
        # tail: release attention PSUM, out-projections nt 4..7 entirely in
        # PSUM (8 banks), one batched Newton rsqrt, direct-PSUM LN applies
        psO.release()
        psE.release()
        psDT = tc.alloc_tile_pool(name="psDT", bufs=6, space="PSUM", side="right")
        halves4, mva = [], mvp.tile([128, 2, 4], F32, tag="mva", bufs=1)
        for j, nt in enumerate(range(4, NT)):
            for hf in range(2):
                ds_ = slice(hf * 512, (hf + 1) * 512)
                pool = psD if j == 0 else psDT
                ps = pool.tile([128, 512], F32, tag="psD" if j == 0 else "psDT")
                for dk in range(DT):
                    nc.tensor.matmul(
                        ps,
                        aoT[:, dk, nt * 128:(nt + 1) * 128],
                        wo_t[dk][:, ds_],
                        start=(dk == 0),
                        stop=(dk == DT - 1),
                    )
                halves4.append(ps)
            stats = stp.tile([128, 2, 6], F32, tag="stats")
            for g in range(2):
                nc.vector.bn_stats(stats[:, g, :], halves4[2 * j + g])
            nc.vector.bn_aggr(mva[:, :, j], stats)
        s4 = mvp.tile([128, 4], F32, tag="s4", bufs=1)
        for half in range(2):
            sl = slice(2 * half, 2 * half + 2)
            wv4 = mvp.tile([128, 2], F32, tag="wv4")
            nc.vector.tensor_scalar(
                out=wv4, in0=mva[:, 1, sl], scalar1=EPS, scalar2=None,
                op0=mybir.AluOpType.add,
            )
            hw4 = mvp.tile([128, 2], F32, tag="hw4")
            nc.vector.tensor_scalar_mul(hw4, wv4, 0.5)
            st = mvp.tile([128, 2], F32, tag="st")
            nc.vector.tensor_scalar_max(st, wv4, 0.03)
            rs4 = mvp.tile([128, 2], F32, tag="rs4")
            d4 = mvp.tile([128, 2], F32, tag="d4")
            for _ in range(4):
                nc.vector.reciprocal(rs4, st)
                nc.vector.tensor_mul(d4, hw4, rs4)
                nc.vector.scalar_tensor_tensor(
                    out=st, in0=st, scalar=0.5, in1=d4,
                    op0=mybir.AluOpType.mult, op1=mybir.AluOpType.add,
                )
            nc.vector.reciprocal(s4[:, sl], st)
        for j, nt in enumerate(range(4, NT)):
            z = finp.tile([128, D], F32, tag="z")
            for g in range(2):
                ds_ = slice(g * 512, (g + 1) * 512)
                t = finp.tile([128, 512], F32, tag="t2", bufs=2)
                nc.vector.scalar_tensor_tensor(
                    out=t, in0=halves4[2 * j + g], scalar=mva[:, 0, j:j + 1],
                    in1=g_bc[:, ds_],
                    op0=mybir.AluOpType.subtract, op1=mybir.AluOpType.mult,
                )
                nc.vector.scalar_tensor_tensor(
                    out=z[:, ds_], in0=t, scalar=s4[:, j:j + 1], in1=b_bc[:, ds_],
                    op0=mybir.AluOpType.mult, op1=mybir.AluOpType.add,
                )
            nc.sync.dma_start(out_d[nt * 128:(nt + 1) * 128, :], z)

        for p in (psDT, psD, finp, wop, att, epsp, qkv, mvp, stp, const):
            p.release()

    nc.compile()
    return nc


_NC_CACHE = None


def _get_nc():
    global _NC_CACHE
    if _NC_CACHE is None:
        _NC_CACHE = build_program()
    return _NC_CACHE


def _host_prep(inputs):
    import ml_dtypes
    wire = ml_dtypes.bfloat16
    f64 = np.float64
    Wq = inputs["Wq"].astype(f64)
    Wk = inputs["Wk"].astype(f64)
    Wv = inputs["Wv"].astype(f64)

    wq = (inputs["nq_g"].astype(f64)[:, None] * Wq).astype(wire)
    wk = (inputs["nk_g"].astype(f64)[:, None] * Wk).astype(wire)
    wv = (inputs["nv_g"].astype(f64)[:, None] * Wv).astype(wire)
    wo = np.ascontiguousarray(inputs["Wo"].astype(f64)).astype(wire)
    bq = (inputs["nq_b"].astype(f64) @ Wq + inputs["bq"].astype(f64)).astype(np.float32)
    bk = (inputs["nk_b"].astype(f64) @ Wk + inputs["bk"].astype(f64)).astype(np.float32)
    bv = (inputs["nv_b"].astype(f64) @ Wv + inputs["bv"].astype(f64)).astype(np.float32)

    # rope tables
    freqs = (1.0 / THETA ** (np.arange(0, DH, 2, dtype=np.float32) / DH)).astype(
        np.float32
    )
    t = np.arange(N, dtype=np.float32)
    ang = np.outer(t, freqs).astype(np.float64)  # [N, 32]
    cos_t = np.cos(ang).astype(np.float32)
    sin_t = np.sin(ang).astype(np.float32)
    p = np.arange(128)
    i_of_p = (p % 64) // 2
    cosf = np.ascontiguousarray(cos_t[:, i_of_p].T).astype(wire)  # [128, N]
    sgn = np.where(p % 2 == 0, -1.0, 1.0).astype(np.float32)
    sinf = np.ascontiguousarray(sin_t[:, i_of_p].T * sgn[:, None]).astype(wire)

    def btab(b):
        tab = np.zeros((128, DT), np.float32)
        for td in range(DT):
            tab[:, td] = b[td * 128 + p]
        return tab

    rows = np.stack(
        [bv, inputs["ln_g"].astype(np.float32), inputs["ln_b"].astype(np.float32)]
    )

    return {
        "wq": wq, "wk": wk, "wv": wv, "wo": wo,
        "cosf": cosf, "sinf": sinf,
        "bqa": btab(bq), "bka": btab(bk),
        "rows": rows.astype(np.float32),
    }


def run(inputs, trace=False, tmpdir=None):
    nc = _get_nc()
    shared = _host_prep(inputs)
    x = np.asarray(inputs["x"], np.float32)
    src = np.asarray(inputs["source"], np.float32)
    in_maps = [
        {"x": np.ascontiguousarray(x[c]), "src": np.ascontiguousarray(src[c]), **shared}
        for c in range(B)
    ]
    res = run_bass_kernel_spmd(nc, in_maps, list(range(B)), trace=trace, tmpdir=tmpdir)
    out = np.stack([res.results[c]["out"] for c in range(B)]).astype(np.float32)
    return out, res


def kernel(**inputs):
    return run(inputs)[0]


# revision 33
# speedup vs baseline: 1.0598x; 1.0598x over previous
"""Trainium2 Bass kernel for nn_AttentionLayer (B=8, N=1024, D=1024, H=16).

Sharding: data-parallel over batch -- one batch element per NeuronCore (8 cores).

Per-core pipeline (restructured for cross-phase overlap; bf16 operands, fp32 PSUM):
  A:  LayerNorm x / source in natural layout, PE-transpose -> snT/xnT [D, tok] bf16.
  V:  v natural via xnT-stationary matmul, ones column per head (early, so the
      attention sweep can consume groups as soon as q/k slices appear).
  Sweep hf=0 (head-pair inner): per pair p: k-proj td=p, q-proj td=p, then
      scoresT -> exp (ACT, the critical engine) -> attn@v with ones-column
      denominator -> normalize via DRAM-bounce reciprocal broadcast.
  Sweep hf=1: same groups (q/k done), with out-projection token tiles 0..3
      interleaved as PE filler; D nt4..7 at the tail.
  Final LN: rstd = Exp(-0.5*Ln(var+eps)) -- stays in the ln/exp ACT table set,
      no table thrash against the sweep's exp; apply = two scalar_tensor_tensor.
"""

import numpy as np

import concourse.bass as bass
import concourse.tile as tile
from concourse import bacc, mybir
from concourse.bass_utils import run_bass_kernel_spmd
from concourse.masks import make_identity

F32 = mybir.dt.float32
BF16 = mybir.dt.bfloat16
MM_DT = BF16   # weights / qT / kT / aoT
EPS_DT = BF16  # exp output + v_sb (attnv operands)

B, N, D, H = 8, 1024, 1024, 16
DH = D // H  # 64
EPS = 1e-5
THETA = 10000.0
NT = N // 128  # 8 token tiles
DT = D // 128  # 8 channel tiles
SCALE = float(DH) ** -0.5

_PAIRSWAP = []
for _i in range(16):
    _PAIRSWAP += [2 * _i + 1, 2 * _i]


def build_program():
    nc = bacc.Bacc("TRN2", target_bir_lowering=False, debug=False)

    x_d = nc.dram_tensor("x", [N, D], F32, kind="ExternalInput").ap()
    s_d = nc.dram_tensor("src", [N, D], F32, kind="ExternalInput").ap()
    wq_d = nc.dram_tensor("wq", [D, D], MM_DT, kind="ExternalInput").ap()
    wk_d = nc.dram_tensor("wk", [D, D], MM_DT, kind="ExternalInput").ap()
    wv_d = nc.dram_tensor("wv", [D, D], MM_DT, kind="ExternalInput").ap()
    wo_d = nc.dram_tensor("wo", [D, D], MM_DT, kind="ExternalInput").ap()
    cosf_d = nc.dram_tensor("cosf", [128, N], MM_DT, kind="ExternalInput").ap()
    sinf_d = nc.dram_tensor("sinf", [128, N], MM_DT, kind="ExternalInput").ap()
    # bias tables [128, DT] (col t = do-tile t)
    bqa_d = nc.dram_tensor("bqa", [128, DT], F32, kind="ExternalInput").ap()
    bka_d = nc.dram_tensor("bka", [128, DT], F32, kind="ExternalInput").ap()
    # row vectors for partition-broadcast loads: bv', ln_g, ln_b
    rows_d = nc.dram_tensor("rows", [3, D], F32, kind="ExternalInput").ap()

    out_d = nc.dram_tensor("out", [N, D], F32, kind="ExternalOutput").ap()
    # bounce rows: [idx] raw sums, [32+idx] reciprocals (reshaped 128-wide)
    rb_d = nc.dram_tensor("rbounce", [H * 4, 512], F32).ap()

    def bcast_row(r):
        return bass.AP(
            tensor=rows_d.tensor, offset=rows_d.offset + r * D, ap=[[0, 128], [1, D]]
        )

    with tile.TileContext(nc) as tc:
        # ---------- pools ----------
        # left stack: whole-kernel lifetime
        const = tc.alloc_tile_pool(name="const", bufs=1, side="left")
        stp = tc.alloc_tile_pool(name="stp", bufs=4, side="left")
        mvp = tc.alloc_tile_pool(name="mvp", bufs=4, side="left")
        qkv = tc.alloc_tile_pool(name="qkv", bufs=1, side="left")
        psP = tc.alloc_tile_pool(name="psP", bufs=2, space="PSUM", side="left")
        # right stack: phase-scoped (alloc in reverse release order)
        wbp = tc.alloc_tile_pool(name="wbp", bufs=16, side="right")
        rope = tc.alloc_tile_pool(name="rope", bufs=2, side="right")
        ntp = tc.alloc_tile_pool(name="ntp", bufs=1, side="right")
        xnp = tc.alloc_tile_pool(name="xnp", bufs=3, side="right")
        ldp = tc.alloc_tile_pool(name="ldp", bufs=4, side="right")
        psT = tc.alloc_tile_pool(name="psT", bufs=4, space="PSUM", side="right")

        # ---- constants (DMAs issued after the first x tiles; see below)
        ident = const.tile([128, 128], F32)
        make_identity(nc, ident)
        eps_t = const.tile([128, 1], F32)
        nc.vector.memset(eps_t, EPS)
        ones128 = const.tile([128, 128], F32)
        nc.vector.memset(ones128, 1.0)
        cosf = const.tile([128, N], MM_DT)
        sinf = const.tile([128, N], MM_DT)
        bqa = const.tile([128, DT], F32)
        bka = const.tile([128, DT], F32)
        bv_bc = const.tile([128, D], F32)

        # ---- persistent attention operands
        qT = qkv.tile([128, DT, N], MM_DT, tag="qT")
        kT = qkv.tile([128, DT, N], MM_DT, tag="kT")
        v_sb = qkv.tile([128, NT, H, 65], EPS_DT, tag="v_sb")
        aoT = qkv.tile([128, DT, N], MM_DT, tag="aoT")
        nc.vector.tensor_copy(
            v_sb[:, :, :, 64:65],
            ones128.rearrange("p (a b c) -> p a b c", a=NT, b=H, c=1),
        )

        def ln_tile(xt, out_ap):
            """LayerNorm [128, D] over free dim -> out_ap (pre-norm phase, Sqrt set)."""
            stats = stp.tile([128, 2, 6], F32, tag="stats")
            for g in range(2):
                nc.vector.bn_stats(stats[:, g, :], xt[:, g * 512:(g + 1) * 512])
            mv = mvp.tile([128, 2], F32, tag="mv")
            nc.vector.bn_aggr(mv, stats)
            nc.scalar.activation(
                mv[:, 1:2], mv[:, 1:2], mybir.ActivationFunctionType.Sqrt, bias=eps_t
            )
            nc.vector.reciprocal(mv[:, 1:2], mv[:, 1:2])
            nc.vector.tensor_scalar(
                out=out_ap,
                in0=xt,
                scalar1=mv[:, 0:1],
                scalar2=mv[:, 1:2],
                op0=mybir.AluOpType.subtract,
                op1=mybir.AluOpType.mult,
            )

        def ln_transpose(src_ap, dstT):
            """dstT [128, DT, N] bf16 = transpose of LN(src)."""
            for t in range(NT):
                xt = ldp.tile([128, D], F32, tag="ld")
                nc.sync.dma_start(xt, src_ap[t * 128:(t + 1) * 128, :])
                xn = xnp.tile([128, D], F32, tag="xn")
                ln_tile(xt, xn)
                for d in range(DT):
                    pt = psT.tile([128, 128], F32, tag="psT")
                    nc.tensor.transpose(pt, xn[:, d * 128:(d + 1) * 128], ident)
                    nc.scalar.copy(dstT[:, d, t * 128:(t + 1) * 128], pt)

        def load_w(w_d):
            tiles = []
            for dk in range(DT):
                wt = wbp.tile([128, D], MM_DT, tag="w")
                nc.sync.dma_start(wt, w_d[dk * 128:(dk + 1) * 128, :])
                tiles.append(wt)
            return tiles

        def qk_proj_td(w_tiles, srcT, dstT, ba, td, hfs=(0, 1)):
            """dstT[:, td, hfs] = RoPE(W.T @ srcT + bias) for one do-tile.

            u = psum + bias (partition pair-swap of u carries the bias swap);
            out = u*cos + swap(u)*sin, all-bf16 on DVE after the biased copy.
            """
            for hf in hfs:
                ns = slice(hf * 512, (hf + 1) * 512)
                ps = psP.tile([128, 512], F32, tag="psP")
                for dk in range(DT):
                    nc.tensor.matmul(
                        ps,
                        w_tiles[dk][:, td * 128:(td + 1) * 128],
                        srcT[:, dk, ns],
                        start=(dk == 0),
                        stop=(dk == DT - 1),
                    )
                u = rope.tile([128, 512], MM_DT, tag="u")
                nc.vector.tensor_scalar(
                    out=u, in0=ps, scalar1=ba[:, td:td + 1], scalar2=None,
                    op0=mybir.AluOpType.add,
                )
                t1 = rope.tile([128, 512], MM_DT, tag="t1")
                nc.vector.tensor_mul(t1, u, cosf[:, ns])
                qs = rope.tile([128, 512], MM_DT, tag="qs")
                nc.vector.stream_shuffle(qs, u, _PAIRSWAP)
                t2 = rope.tile([128, 512], MM_DT, tag="t2")
                nc.vector.tensor_mul(t2, qs, sinf[:, ns])
                nc.vector.tensor_add(dstT[:, td, ns], t1, t2)

        # ============ Prologue: x & src LN/transpose with V interleaved ============
        # V group t depends only on x-tile t's transposes; src tiles alternate so
        # the DVE LayerNorm pipeline feeds the PE without gaps.
        xnT = ntp.tile([128, DT, N], MM_DT, tag="xnT")
        snT = ntp.tile([128, DT, N], MM_DT, tag="snT")
        xts, sts = [], []
        wv_t = wk_t = None
        for t in range(NT):
            xt = ldp.tile([128, D], F32, tag="ld")
            nc.sync.dma_start(xt, x_d[t * 128:(t + 1) * 128, :])
            xts.append(xt)
            st = ldp.tile([128, D], F32, tag="ld")
            nc.sync.dma_start(st, s_d[t * 128:(t + 1) * 128, :])
            sts.append(st)
            if t == 1:
                nc.sync.dma_start(cosf, cosf_d)
                nc.sync.dma_start(sinf, sinf_d)
                nc.sync.dma_start(bqa, bqa_d)
                nc.sync.dma_start(bka, bka_d)
                nc.sync.dma_start(bv_bc, bcast_row(0))
                wv_t = load_w(wv_d)
            if t == 3:
                wk_t = load_w(wk_d)

        def transpose_tile(xn, dstT, t):
            for d in range(DT):
                pt = psT.tile([128, 128], F32, tag="psT")
                nc.tensor.transpose(pt, xn[:, d * 128:(d + 1) * 128], ident)
                nc.scalar.copy(dstT[:, d, t * 128:(t + 1) * 128], pt)

        for t in range(NT):
            xn = xnp.tile([128, D], F32, tag="xn")
            ln_tile(xts[t], xn)
            transpose_tile(xn, xnT, t)
            for hf in range(2):
                ds_ = slice(hf * 512, (hf + 1) * 512)
                ps = psP.tile([128, 512], F32, tag="psP")
                for dk in range(DT):
                    nc.tensor.matmul(
                        ps,
                        xnT[:, dk, t * 128:(t + 1) * 128],
                        wv_t[dk][:, ds_],
                        start=(dk == 0),
                        stop=(dk == DT - 1),
                    )
                nc.vector.tensor_add(
                    v_sb[:, t, 8 * hf:8 * hf + 8, 0:64],
                    ps.rearrange("p (j d) -> p j d", j=8),
                    bv_bc[:, ds_].rearrange("p (j d) -> p j d", j=8),
                )
            sn = xnp.tile([128, D], F32, tag="xn")
            ln_tile(sts[t], sn)
            transpose_tile(sn, snT, t)

        wq_t = load_w(wq_d)  # reuses wv slots after V completes
        # q/k projections for the first two pairs (k both halves, q half 0)
        for pr in range(2):
            qk_proj_td(wk_t, xnT, kT, bka, pr, (0, 1))
            qk_proj_td(wq_t, snT, qT, bqa, pr, (0,))
        for p in (psT, ldp, xnp):
            p.release()

        psE = tc.alloc_tile_pool(name="psE", bufs=2, space="PSUM", side="right")
        psO = tc.alloc_tile_pool(name="psO", bufs=2, space="PSUM", side="right")
        epsp = tc.alloc_tile_pool(name="epsp", bufs=16, side="left")
        att = tc.alloc_tile_pool(name="att", bufs=3, side="left")

        def attn_scores(pair, hf, lo, hi, eps_tiles):
            ns = slice(hf * 512, (hf + 1) * 512)
            for mb in range(lo, hi):
                pse = psE.tile([128, 1024], F32, tag="psE")
                nc.tensor.matmul(
                    pse[:, 0:512],
                    kT[0:64, pair, mb * 128:(mb + 1) * 128],
                    qT[0:64, pair, ns],
                    start=True, stop=True,
                )
                nc.tensor.matmul(
                    pse[:, 512:1024],
                    kT[64:128, pair, mb * 128:(mb + 1) * 128],
                    qT[64:128, pair, ns],
                    start=True, stop=True,
                )
                et = epsp.tile([128, 2, 512], EPS_DT, tag="eps")
                nc.scalar.activation(
                    et, pse, mybir.ActivationFunctionType.Exp, scale=SCALE,
                )
                eps_tiles.append(et)

        def attn_av_norm(pair, hf, eps_tiles, mid=None, fast=False):
            he, ho = 2 * pair, 2 * pair + 1
            ns = slice(hf * 512, (hf + 1) * 512)
            pso_e = psO.tile([128, 512], F32, tag="psO")
            pso_o = psO.tile([128, 512], F32, tag="psO")
            for mb in range(NT):
                e_mb = eps_tiles[mb]
                nc.tensor.matmul(
                    pso_e[0:65, :], v_sb[:, mb, he, :], e_mb[:, 0, :],
                    start=(mb == 0), stop=(mb == NT - 1),
                )
                nc.tensor.matmul(
                    pso_o[0:65, :], v_sb[:, mb, ho, :], e_mb[:, 1, :],
                    start=(mb == 0), stop=(mb == NT - 1),
                )
                if mid is not None and mb in (1, 3, 5):
                    mid((mb - 1) // 2)
            # normalize: merged-parity reciprocal via DRAM-bounce broadcast
            # (one spread/recip round trip for both heads; queue alternates)
            dq = nc.gpsimd if (pair + hf) % 2 == 0 else nc.sync
            dq2 = nc.sync if (pair + hf) % 2 == 0 else nc.gpsimd
            idx = 4 * pair + 2 * hf  # rows idx, idx+1
            r_e = att.tile([128, 512], F32, tag="r_sb")
            nc.vector.tensor_copy(r_e[0:65, :], pso_e[0:65, :])
            r_o = att.tile([128, 512], F32, tag="r_sb2")
            nc.vector.tensor_copy(r_o[0:65, :], pso_o[0:65, :])
            dq.dma_start(rb_d[idx:idx + 1, :], r_e[64:65, :])
            dq.dma_start(rb_d[idx + 1:idx + 2, :], r_o[64:65, :])
            rt = att.tile([128, 8], F32, tag="rt")
            dq.dma_start(
                rt,
                bass.AP(
                    tensor=rb_d.tensor,
                    offset=rb_d.offset + idx * 512,
                    ap=[[8, 128], [1, 8]],
                ),
            )
            nc.vector.reciprocal(rt, rt)
            dq.dma_start(
                bass.AP(
                    tensor=rb_d.tensor,
                    offset=rb_d.offset + (32 + idx) * 512,
                    ap=[[8, 128], [1, 8]],
                ),
                rt,
            )
            bc_e = att.tile([64, 512], F32, tag="bc")
            dq.dma_start(
                bc_e,
                bass.AP(
                    tensor=rb_d.tensor,
                    offset=rb_d.offset + (32 + idx) * 512,
                    ap=[[0, 64], [1, 512]],
                ),
            )
            bc_o = att.tile([64, 512], F32, tag="bc2")
            dq.dma_start(
                bc_o,
                bass.AP(
                    tensor=rb_d.tensor,
                    offset=rb_d.offset + (33 + idx) * 512,
                    ap=[[0, 64], [1, 512]],
                ),
            )
            nc.vector.tensor_mul(aoT[0:64, pair, ns], r_e[0:64, :], bc_e)
            tmp = att.tile([64, 512], MM_DT, tag="tmp")
            nc.vector.tensor_mul(tmp, r_o[0:64, :], bc_o)
            dq2.dma_start(aoT[64:128, pair, ns], tmp)

        wop = finp = psD = None
        wo_t = []
        g_bc = b_bc = None

        def late_pools():
            nonlocal wop, finp, psD, g_bc, b_bc
            for p in (ntp, rope, wbp):
                p.release()
            psP.release()
            wop = tc.alloc_tile_pool(name="wop", bufs=8, side="right")
            finp = tc.alloc_tile_pool(name="finp", bufs=3, side="right")
            psD = tc.alloc_tile_pool(name="psD", bufs=2, space="PSUM", side="left")
            for dk in range(DT):
                wt = wop.tile([128, D], MM_DT, tag="wo")
                nc.sync.dma_start(wt, wo_d[dk * 128:(dk + 1) * 128, :])
                wo_t.append(wt)
            g_bc = wop.tile([128, D], F32, tag="g_bc", bufs=1)
            nc.sync.dma_start(g_bc, bcast_row(1))
            b_bc = wop.tile([128, D], F32, tag="b_bc", bufs=1)
            nc.sync.dma_start(b_bc, bcast_row(2))

        def out_proj(nt):
            halves = []
            for hf in range(2):
                ds_ = slice(hf * 512, (hf + 1) * 512)
                ps = psD.tile([128, 512], F32, tag="psD")
                for dk in range(DT):
                    nc.tensor.matmul(
                        ps,
                        aoT[:, dk, nt * 128:(nt + 1) * 128],
                        wo_t[dk][:, ds_],
                        start=(dk == 0),
                        stop=(dk == DT - 1),
                    )
                fin = finp.tile([128, 512], F32, tag="t")
                nc.vector.tensor_copy(fin, ps)
                halves.append(fin)
            stats = stp.tile([128, 2, 6], F32, tag="stats")
            for g in range(2):
                nc.vector.bn_stats(stats[:, g, :], halves[g])
            mv = mvp.tile([128, 2], F32, tag="mv")
            nc.vector.bn_aggr(mv, stats)
            # rstd = 1/sqrt(var+eps) DVE-only (Newton sqrt; no ACT table switch)
            w_t = mvp.tile([128, 1], F32, tag="w")
            nc.vector.tensor_scalar(
                out=w_t, in0=mv[:, 1:2], scalar1=EPS, scalar2=None,
                op0=mybir.AluOpType.add,
            )
            hw_t = mvp.tile([128, 1], F32, tag="hw")
            nc.vector.tensor_scalar_mul(hw_t, w_t, 0.5)
            s_t = mvp.tile([128, 1], F32, tag="s")
            nc.vector.tensor_scalar_max(s_t, w_t, 0.03)
            rs_t = mvp.tile([128, 1], F32, tag="rs")
            d_t = mvp.tile([128, 1], F32, tag="d")
            for _ in range(4):
                nc.vector.reciprocal(rs_t, s_t)
                nc.vector.tensor_mul(d_t, hw_t, rs_t)
                nc.vector.scalar_tensor_tensor(
                    out=s_t, in0=s_t, scalar=0.5, in1=d_t,
                    op0=mybir.AluOpType.mult, op1=mybir.AluOpType.add,
                )
            nc.vector.reciprocal(mv[:, 1:2], s_t)
            z = finp.tile([128, D], F32, tag="z")
            for g in range(2):
                ds_ = slice(g * 512, (g + 1) * 512)
                t = finp.tile([128, 512], F32, tag="t")
                nc.vector.scalar_tensor_tensor(
                    out=t, in0=halves[g], scalar=mv[:, 0:1], in1=g_bc[:, ds_],
                    op0=mybir.AluOpType.subtract, op1=mybir.AluOpType.mult,
                )
                nc.vector.scalar_tensor_tensor(
                    out=z[:, ds_], in0=t, scalar=mv[:, 1:2], in1=b_bc[:, ds_],
                    op0=mybir.AluOpType.mult, op1=mybir.AluOpType.add,
                )
            nc.sync.dma_start(out_d[nt * 128:(nt + 1) * 128, :], z)

        # ============ Pipelined group sweep (hf0/hf1 interleaved) ============
        # Groups ordered so projection fillers spread evenly; scores for group
        # g+1 are injected inside g's attnv chain so the exp stream never
        # drains. Out-projection token tiles ride the last four iterations.
        order = [(0, 0), (1, 0), (2, 0), (3, 0), (0, 1), (4, 0), (1, 1), (5, 0),
                 (2, 1), (6, 0), (3, 1), (7, 0), (4, 1), (5, 1), (6, 1), (7, 1)]
        fillers = {
            0: [("k0", 2), ("k1", 2), ("q0", 2)],
            1: [("k0", 3), ("k1", 3), ("q0", 3)],
            2: [("q1", 0), ("k0", 4)],
            3: [("k1", 4), ("q0", 4)],
            4: [("k0", 5), ("q1", 1)],
            5: [("k1", 5), ("q0", 5)],
            6: [("k0", 6), ("q1", 2)],
            7: [("k1", 6), ("q0", 6)],
            8: [("k0", 7), ("q1", 3)],
            9: [("k1", 7), ("q0", 7)],
            10: [("q1", 4), ("q1", 5)],
            11: [("q1", 6), ("q1", 7)],
        }
        douts = {12: 0, 13: 1, 14: 2, 15: 3}
        store = {}
        store[order[0]] = []
        attn_scores(order[0][0], order[0][1], 0, NT, store[order[0]])
        for i, g in enumerate(order):
            nxt = order[i + 1] if i + 1 < len(order) else None
            if nxt is not None:
                store[nxt] = []

                def mid(j, nx=nxt):
                    attn_scores(nx[0], nx[1], j, j + 1, store[nx])
            else:
                mid = None
            attn_av_norm(g[0], g[1], store.pop(g), mid=mid, fast=(g == (7, 1)))
            if nxt is not None:
                attn_scores(nxt[0], nxt[1], 3, NT, store[nxt])
            for kind, td in fillers.get(i, ()):
                if kind == "k0":
                    qk_proj_td(wk_t, xnT, kT, bka, td, (0,))
                elif kind == "k1":
                    qk_proj_td(wk_t, xnT, kT, bka, td, (1,))
                elif kind == "q0":
                    qk_proj_td(wq_t, snT, qT, bqa, td, (0,))
                else:
                    qk_proj_td(wq_t, snT, qT, bqa, td, (1,))
            if i == 11:
                late_pools()
            if i in douts:
                out_proj(douts[i])
        # tail: release attention PSUM, out-projections nt 4..7 entirely in
        # PSUM (8 banks), one batched Newton rsqrt, direct-PSUM LN applies
        psO.release()
        psE.release()
        psDT = tc.alloc_tile_pool(name="psDT", bufs=6, space="PSUM", side="right")
        halves4, mva = [], mvp.tile([128, 2, 4], F32, tag="mva", bufs=1)
        for j, nt in enumerate(range(4, NT)):
            for hf in range(2):
                ds_ = slice(hf * 512, (hf + 1) * 512)
                pool = psD if j == 0 else psDT
                ps = pool.tile([128, 512], F32, tag="psD" if j == 0 else "psDT")
                for dk in range(DT):
                    nc.tensor.matmul(
                        ps,
                        aoT[:, dk, nt * 128:(nt + 1) * 128],
                        wo_t[dk][:, ds_],
                        start=(dk == 0),
                        stop=(dk == DT - 1),
                    )
                halves4.append(ps)
            stats = stp.tile([128, 2, 6], F32, tag="stats")
            for g in range(2):
                nc.vector.bn_stats(stats[:, g, :], halves4[2 * j + g])
            nc.vector.bn_aggr(mva[:, :, j], stats)
        wv4 = mvp.tile([128, 4], F32, tag="wv4", bufs=1)
        nc.vector.tensor_scalar(
            out=wv4, in0=mva[:, 1, :], scalar1=EPS, scalar2=None,
            op0=mybir.AluOpType.add,
        )
        hw4 = mvp.tile([128, 4], F32, tag="hw4", bufs=1)
        nc.vector.tensor_scalar_mul(hw4, wv4, 0.5)
        s4 = mvp.tile([128, 4], F32, tag="s4", bufs=1)
        nc.vector.tensor_scalar_max(s4, wv4, 0.03)
        rs4 = mvp.tile([128, 4], F32, tag="rs4", bufs=1)
        d4 = mvp.tile([128, 4], F32, tag="d4", bufs=1)
        for _ in range(4):
            nc.vector.reciprocal(rs4, s4)
            nc.vector.tensor_mul(d4, hw4, rs4)
            nc.vector.scalar_tensor_tensor(
                out=s4, in0=s4, scalar=0.5, in1=d4,
                op0=mybir.AluOpType.mult, op1=mybir.AluOpType.add,
            )
        nc.vector.reciprocal(s4, s4)
        for j, nt in enumerate(range(4, NT)):
            z = finp.tile([128, D], F32, tag="z")
            for g in range(2):
                ds_ = slice(g * 512, (g + 1) * 512)
                t = finp.tile([128, 512], F32, tag="t2", bufs=2)
                nc.vector.scalar_tensor_tensor(
                    out=t, in0=halves4[2 * j + g], scalar=mva[:, 0, j:j + 1],
                    in1=g_bc[:, ds_],
                    op0=mybir.AluOpType.subtract, op1=mybir.AluOpType.mult,
                )
                nc.vector.scalar_tensor_tensor(
                    out=z[:, ds_], in0=t, scalar=s4[:, j:j + 1], in1=b_bc[:, ds_],
                    op0=mybir.AluOpType.mult, op1=mybir.AluOpType.add,
                )
            nc.sync.dma_start(out_d[nt * 128:(nt + 1) * 128, :], z)

        for p in (psDT, psD, finp, wop, att, epsp, qkv, mvp, stp, const):
            p.release()

    nc.compile()
    return nc


_NC_CACHE = None


def _get_nc():
    global _NC_CACHE
    if _NC_CACHE is None:
        _NC_CACHE = build_program()
    return _NC_CACHE


def _host_prep(inputs):
    import ml_dtypes
    wire = ml_dtypes.bfloat16
    f64 = np.float64
    Wq = inputs["Wq"].astype(f64)
    Wk = inputs["Wk"].astype(f64)
    Wv = inputs["Wv"].astype(f64)

    wq = (inputs["nq_g"].astype(f64)[:, None] * Wq).astype(wire)
    wk = (inputs["nk_g"].astype(f64)[:, None] * Wk).astype(wire)
    wv = (inputs["nv_g"].astype(f64)[:, None] * Wv).astype(wire)
    wo = np.ascontiguousarray(inputs["Wo"].astype(f64)).astype(wire)
    bq = (inputs["nq_b"].astype(f64) @ Wq + inputs["bq"].astype(f64)).astype(np.float32)
    bk = (inputs["nk_b"].astype(f64) @ Wk + inputs["bk"].astype(f64)).astype(np.float32)
    bv = (inputs["nv_b"].astype(f64) @ Wv + inputs["bv"].astype(f64)).astype(np.float32)

    # rope tables
    freqs = (1.0 / THETA ** (np.arange(0, DH, 2, dtype=np.float32) / DH)).astype(
        np.float32
    )
    t = np.arange(N, dtype=np.float32)
    ang = np.outer(t, freqs).astype(np.float64)  # [N, 32]
    cos_t = np.cos(ang).astype(np.float32)
    sin_t = np.sin(ang).astype(np.float32)
    p = np.arange(128)
    i_of_p = (p % 64) // 2
    cosf = np.ascontiguousarray(cos_t[:, i_of_p].T).astype(wire)  # [128, N]
    sgn = np.where(p % 2 == 0, -1.0, 1.0).astype(np.float32)
    sinf = np.ascontiguousarray(sin_t[:, i_of_p].T * sgn[:, None]).astype(wire)

    def btab(b):
        tab = np.zeros((128, DT), np.float32)
        for td in range(DT):
            tab[:, td] = b[td * 128 + p]
        return tab

    rows = np.stack(
        [bv, inputs["ln_g"].astype(np.float32), inputs["ln_b"].astype(np.float32)]
    )

    return {
        "wq": wq, "wk": wk, "wv": wv, "wo": wo,
        "cosf": cosf, "sinf": sinf,
        "bqa": btab(bq), "bka": btab(bk),
        "rows": rows.astype(np.float32),
    }


def run(inputs, trace=False, tmpdir=None):
    nc = _get_nc()
    shared = _host_prep(inputs)
    x = np.asarray(inputs["x"], np.float32)
    src = np.asarray(inputs["source"], np.float32)
    in_maps = [
        {"x": np.ascontiguousarray(x[c]), "src": np.ascontiguousarray(src[c]), **shared}
        for c in range(B)
    ]
    res = run_bass_kernel_spmd(nc, in_maps, list(range(B)), trace=trace, tmpdir=tmpdir)
    out = np.stack([res.results[c]["out"] for c in range(B)]).astype(np.float32)
    return out, res


def kernel(**inputs):
    return run(inputs)[0]
